# revision 5
# baseline (speedup 1.0000x reference)
"""Multi-head attention (B=2, S=2048, D=1024, H=16, d_k=d_v=64) on 8 trn2 cores.

Sharding: core c handles batch b = c//4 and heads [4j, 4j+4) where j = c%4
(data parallel over B, tensor parallel over heads). Each core:

  1. Projects its 4 heads' Q/K/V in feature-major ("transposed") layout
     directly from host-pre-transposed Q^T/K^T/V^T inputs:
         QhT = Wq_h^T @ Q^T   [64, 2048]   (d_k on partitions)
     so both score layouts come straight off the tensor engine with no
     on-chip transposes.
  2. Pass 1 (k-major): scoresT[k,q] tiles -> exp on ACT -> AV matmuls with a
     ones-column appended to V, so PSUM row 64 accumulates the softmax
     denominator for free (no partition-direction reductions).
  3. Pass 2 (q-major): recompute scores with Q as the stationary operand and
     apply exp(s/8 - log(denom)) in one ACT op (bias = -log denom, obtained
     by a tiny PE transpose of the denominator row) -> normalized attention
     probabilities written straight to DRAM in output layout.
  4. Context is normalized in feature-major form and multiplied by this
     core's 256-row slice of Wo -> partial output [2048, 1024].
  5. ReduceScatter(add) over the 4 cores of the batch group sums the
     partials and hands each core its own 512-row chunk; bias + residual +
     LayerNorm run locally; host concatenates the chunks.

Matmul operands are stored as float32r (single-pass fp32 on the PE array,
~4e-4 relative error, 4x faster than exact fp32).
"""

import numpy as np

import concourse.bass as bass
import concourse.mybir as mybir
import concourse.tile as tile
from concourse import bacc
from concourse.bass_utils import run_bass_kernel_spmd
from concourse.masks import make_identity

F32 = mybir.dt.float32
F32R = mybir.dt.float32r
AF = mybir.ActivationFunctionType
ALU = mybir.AluOpType

B, S, D = 2, 2048, 1024
H, DK, DV = 16, 64, 64
HPC = 4              # heads per core
N_CORES = 8
ROWS = S // 4        # out rows owned per core (ReduceScatter chunk)
LN_EPS = 1e-5
REPLICA_GROUPS = [[0, 1, 2, 3], [4, 5, 6, 7]]

_PROGRAM_CACHE = {}


def _build_program():
    nc = bacc.Bacc(None, target_bir_lowering=False)

    # ---- I/O ----------------------------------------------------------------
    # fp32 numpy buffers feed float32r DRAM tensors bit-identically.
    qT = nc.dram_tensor("qT", [D, S], F32R, kind="ExternalInput")
    kT = nc.dram_tensor("kT", [D, S], F32R, kind="ExternalInput")
    vT = nc.dram_tensor("vT", [D, S], F32R, kind="ExternalInput")
    wq4 = nc.dram_tensor("wq4", [D, HPC * DK], F32R, kind="ExternalInput")
    wk4 = nc.dram_tensor("wk4", [D, HPC * DK], F32R, kind="ExternalInput")
    wv4 = nc.dram_tensor("wv4", [D, HPC * DV], F32R, kind="ExternalInput")
    wo4 = nc.dram_tensor("wo4", [HPC * DV, D], F32R, kind="ExternalInput")
    bq2 = nc.dram_tensor("bq2", [128, 2], F32, kind="ExternalInput")
    bk2 = nc.dram_tensor("bk2", [128, 2], F32, kind="ExternalInput")
    boeff = nc.dram_tensor("boeff", [1, D], F32, kind="ExternalInput")
    gamma_r = nc.dram_tensor("gamma_r", [1, D], F32, kind="ExternalInput")
    beta_r = nc.dram_tensor("beta_r", [1, D], F32, kind="ExternalInput")
    qres = nc.dram_tensor("qres", [ROWS, D], F32, kind="ExternalInput")

    attn_o = nc.dram_tensor("attn_o", [HPC, S, S], F32, kind="ExternalOutput")
    out_o = nc.dram_tensor("out_o", [ROWS, D], F32, kind="ExternalOutput")

    NKT = S // 128   # 16 k tiles of 128
    NQB = S // 512   # 4 q blocks of 512
    NQT = S // 128   # 16 q tiles of 128

    with tile.TileContext(nc) as tc:
        with (
            tc.tile_pool(name="persist", bufs=1) as persist,
            tc.tile_pool(name="ps_big", bufs=2, space="PSUM") as ps_big,     # [128,1024] x2 = 4 banks
            tc.tile_pool(name="ps_ctx", bufs=2, space="PSUM") as ps_ctx,     # [128,512] x2 = 2 banks
            tc.tile_pool(name="ps_misc", bufs=2, space="PSUM") as ps_misc,   # [128,512] x2 = 2 banks
            tc.tile_pool(name="dram", bufs=1, space="DRAM") as dram,
        ):
            # persistent SBUF state
            qhT = [persist.tile([128, S], F32R, name=f"qhT{p}") for p in range(2)]
            khT = [persist.tile([128, S], F32R, name=f"khT{p}") for p in range(2)]
            vh_aug = [persist.tile([128, NKT * (DV + 1)], F32R, name=f"vhaug{h}")
                      for h in range(HPC)]
            ctxn = [persist.tile([128, S], F32R, name=f"ctxn{p}") for p in range(2)]
            ident = persist.tile([128, 128], F32, name="ident")
            make_identity(nc, ident[:])
            ones64 = persist.tile([1, 64], F32, name="ones64")
            nc.vector.memset(ones64[:], 1.0)
            nldpad = persist.tile([128, 512], F32, name="nldpad")
            nc.vector.memset(nldpad[:], 0.0)

            bq_sb = persist.tile([128, 2], F32, name="bq_sb")
            bk_sb = persist.tile([128, 2], F32, name="bk_sb")
            nc.sync.dma_start(out=bq_sb[:], in_=bq2[:])
            nc.sync.dma_start(out=bk_sb[:], in_=bk2[:])

            # ones column (col 64 of each 65-wide chunk) of vh_aug: DVE copy
            # from an f32 ones tile (memset cannot target float32r).
            onecol = persist.tile([128, 1], F32, name="onecol")
            nc.vector.memset(onecol[:], 1.0)
            for h in range(HPC):
                for kt in range(S // 128):
                    nc.vector.tensor_copy(
                        vh_aug[h][:, (DV + 1) * kt + DV:(DV + 1) * kt + DV + 1],
                        onecol[:],
                    )

            # ---- Phase A: projections --------------------------------------
            with (
                tc.tile_pool(name="wpool", bufs=1) as wpool,
                tc.tile_pool(name="inpool", bufs=9) as inpool,
            ):
                wq_sb = wpool.tile([128, 8, HPC * DK], F32R, name="wq_sb")
                wk_sb = wpool.tile([128, 8, HPC * DK], F32R, name="wk_sb")
                wv_sb = wpool.tile([128, 8, HPC * DV], F32R, name="wv_sb")
                nc.sync.dma_start(out=wq_sb[:], in_=wq4.rearrange("(t p) n -> p t n", p=128))
                nc.sync.dma_start(out=wk_sb[:], in_=wk4.rearrange("(t p) n -> p t n", p=128))
                nc.sync.dma_start(out=wv_sb[:], in_=wv4.rearrange("(t p) n -> p t n", p=128))

                def project_qk(src_dram, w_sb, b_sb, dst_pair):
                    tiles = []
                    for t in range(8):
                        it = inpool.tile([128, S], F32R, name=f"in{t}", tag="intile")
                        nc.sync.dma_start(out=it[:], in_=src_dram[128 * t:128 * (t + 1), :])
                        tiles.append(it)
                    for pair in range(2):
                        for sb4 in range(NQB):
                            ps = ps_big.tile([128, 512], F32, name="projps", tag="bigps")
                            for t in range(8):
                                nc.tensor.matmul(
                                    ps[:],
                                    lhsT=w_sb[:, t, 128 * pair:128 * (pair + 1)],
                                    rhs=tiles[t][:, 512 * sb4:512 * (sb4 + 1)],
                                    start=(t == 0), stop=(t == 7),
                                )
                            nc.vector.tensor_scalar_add(
                                out=dst_pair[pair][:, 512 * sb4:512 * (sb4 + 1)],
                                in0=ps[:],
                                scalar1=b_sb[:, pair:pair + 1],
                            )

                project_qk(qT, wq_sb, bq_sb, qhT)
                project_qk(kT, wk_sb, bk_sb, khT)

                # V: natural layout [k, dv] per head, with ones column
                vtiles = []
                for t in range(8):
                    it = inpool.tile([128, S], F32R, name=f"vin{t}", tag="intile")
                    nc.sync.dma_start(out=it[:], in_=vT[128 * t:128 * (t + 1), :])
                    vtiles.append(it)
                for kt in range(NKT):
                    ps = ps_big.tile([128, HPC * DV], F32, name="vps", tag="bigps")
                    for t in range(8):
                        nc.tensor.matmul(
                            ps[:],
                            lhsT=vtiles[t][:, 128 * kt:128 * (kt + 1)],
                            rhs=wv_sb[:, t, :],
                            start=(t == 0), stop=(t == 7),
                        )
                    for h in range(HPC):
                        nc.vector.tensor_copy(
                            vh_aug[h][:, (DV + 1) * kt:(DV + 1) * kt + DV],
                            ps[:, DV * h:DV * (h + 1)],
                        )

            # ---- Phase B: attention ----------------------------------------
            with (
                tc.tile_pool(name="exp_pool", bufs=3) as exp_pool,
                tc.tile_pool(name="attn_pool", bufs=3) as attn_pool,
                tc.tile_pool(name="sm_pool", bufs=4) as sm_pool,
                tc.tile_pool(name="col_pool", bufs=2 * NQT) as col_pool,
            ):
                for h in range(HPC):
                    p, off = h // 2, 64 * (h % 2)
                    nld_cols = []
                    for qb in range(NQB):
                        qsl = slice(512 * qb, 512 * (qb + 1))
                        ctx_ps = ps_ctx.tile([128, 512], F32, name="ctxps", tag="ctxps")
                        for kt2 in range(NKT // 2):
                            sc = ps_big.tile([128, 1024], F32, name="scps", tag="bigps")
                            for half in range(2):
                                kt = 2 * kt2 + half
                                nc.tensor.matmul(
                                    sc[:, 512 * half:512 * (half + 1)],
                                    lhsT=khT[p][off:off + 64, 128 * kt:128 * (kt + 1)],
                                    rhs=qhT[p][off:off + 64, qsl],
                                    start=True, stop=True,
                                )
                            ex = exp_pool.tile([128, 1024], F32R, name="ex", tag="ex")
                            nc.scalar.activation(out=ex[:], in_=sc[:], func=AF.Exp, scale=0.125)
                            for half in range(2):
                                kt = 2 * kt2 + half
                                nc.tensor.matmul(
                                    ctx_ps[0:DV + 1, :],
                                    lhsT=vh_aug[h][:, (DV + 1) * kt:(DV + 1) * (kt + 1)],
                                    rhs=ex[:, 512 * half:512 * (half + 1)],
                                    start=(kt == 0), stop=(kt == NKT - 1),
                                )
                        # denominator handling
                        inv_d = sm_pool.tile([1, 512], F32, name="inv_d", tag="inv_d")
                        nc.vector.reciprocal(out=inv_d[:], in_=ctx_ps[DV:DV + 1, :])
                        nld = sm_pool.tile([1, 512], F32, name="nld", tag="nld")
                        nc.scalar.activation(out=nld[:], in_=inv_d[:], func=AF.Ln)
                        nc.vector.tensor_copy(nldpad[0:1, :], nld[:])
                        for qq in range(4):
                            trp = ps_misc.tile([128, 128], F32, name="trp", tag="miscps")
                            nc.tensor.transpose(trp[:], nldpad[:, 128 * qq:128 * (qq + 1)], ident[:])
                            nldc = col_pool.tile([128, 1], F32, name="nldc", tag="nldc")
                            nc.vector.tensor_copy(nldc[:], trp[:, 0:1])
                            nld_cols.append(nldc)
                        # normalize context (feature-major)
                        rep = ps_misc.tile([64, 512], F32, name="rep", tag="miscps")
                        nc.tensor.matmul(rep[:], lhsT=ones64[:], rhs=inv_d[:], start=True, stop=True)
                        rep_sb = sm_pool.tile([64, 512], F32, name="rep_sb", tag="rep_sb")
                        nc.vector.tensor_copy(rep_sb[:], rep[:])
                        nc.vector.tensor_mul(
                            ctxn[p][off:off + 64, qsl], ctx_ps[0:DV, :], rep_sb[:],
                        )
                    # pass 2: q-major normalized attention -> DRAM
                    for qt in range(NQT):
                        at = attn_pool.tile([128, S], F32, name="at", tag="at")
                        for half2 in range(2):
                            s2 = ps_big.tile([128, 1024], F32, name="s2ps", tag="bigps")
                            for kb in range(2):
                                ko = 1024 * half2 + 512 * kb
                                nc.tensor.matmul(
                                    s2[:, 512 * kb:512 * (kb + 1)],
                                    lhsT=qhT[p][off:off + 64, 128 * qt:128 * (qt + 1)],
                                    rhs=khT[p][off:off + 64, ko:ko + 512],
                                    start=True, stop=True,
                                )
                            nc.scalar.activation(
                                out=at[:, 1024 * half2:1024 * (half2 + 1)],
                                in_=s2[:], func=AF.Exp, scale=0.125,
                                bias=nld_cols[qt][:],
                            )
                        nc.sync.dma_start(
                            out=attn_o[h, 128 * qt:128 * (qt + 1), :], in_=at[:],
                        )

            # ---- Phase C: partial out-projection + ReduceScatter ------------
            cc_in = dram.tile([S, D], F32, name="cc_in")
            cc_out = dram.tile([ROWS, D], F32, name="cc_out")
            with (
                tc.tile_pool(name="wo_pool", bufs=1) as wo_pool,
                tc.tile_pool(name="stage_pool", bufs=3) as stage_pool,
            ):
                wo_sb = wo_pool.tile([128, 2, D], F32R, name="wo_sb")
                nc.sync.dma_start(out=wo_sb[:], in_=wo4.rearrange("(t p) n -> p t n", p=128))
                for qt in range(NQT):
                    op = ps_big.tile([128, 1024], F32, name="opps", tag="bigps")
                    for dmb in range(2):
                        for t in range(2):
                            nc.tensor.matmul(
                                op[:, 512 * dmb:512 * (dmb + 1)],
                                lhsT=ctxn[t][:, 128 * qt:128 * (qt + 1)],
                                rhs=wo_sb[:, t, 512 * dmb:512 * (dmb + 1)],
                                start=(t == 0), stop=(t == 1),
                            )
                    stg = stage_pool.tile([128, D], F32, name="stg", tag="stg")
                    nc.vector.tensor_copy(stg[:], op[:])
                    nc.sync.dma_start(out=cc_in[128 * qt:128 * (qt + 1), :], in_=stg[:])
                nc.gpsimd.collective_compute(
                    "ReduceScatter", ALU.add,
                    replica_groups=REPLICA_GROUPS,
                    ins=[cc_in.opt()], outs=[cc_out.opt()],
                )

            # ---- Phase D: bias + residual + LayerNorm ----------------------
            with (
                tc.tile_pool(name="fin_pool", bufs=4) as fin_pool,
                tc.tile_pool(name="ln_pool", bufs=1) as ln_pool,
                tc.tile_pool(name="lnsm", bufs=4) as lnsm,
            ):
                bo_rep = ln_pool.tile([128, D], F32, name="bo_rep")
                ga_rep = ln_pool.tile([128, D], F32, name="ga_rep")
                be_rep = ln_pool.tile([128, D], F32, name="be_rep")
                nc.gpsimd.dma_start(out=bo_rep[:], in_=boeff.ap().to_broadcast((128, D)))
                nc.gpsimd.dma_start(out=ga_rep[:], in_=gamma_r.ap().to_broadcast((128, D)))
                nc.gpsimd.dma_start(out=be_rep[:], in_=beta_r.ap().to_broadcast((128, D)))
                eps_t = ln_pool.tile([128, 1], F32, name="eps_t")
                nc.vector.memset(eps_t[:], LN_EPS)

                for qt in range(ROWS // 128):
                    ft = fin_pool.tile([128, D], F32, name="ft", tag="ft")
                    nc.sync.dma_start(out=ft[:], in_=cc_out[128 * qt:128 * (qt + 1), :])
                    rs = fin_pool.tile([128, D], F32, name="rs", tag="rs")
                    nc.sync.dma_start(out=rs[:], in_=qres[128 * qt:128 * (qt + 1), :])
                    nc.vector.tensor_add(out=ft[:], in0=ft[:], in1=bo_rep[:])
                    nc.vector.tensor_add(out=ft[:], in0=ft[:], in1=rs[:])
                    stats = lnsm.tile([128, 2, 6], F32, name="stats", tag="stats")
                    fg = ft.rearrange("p (g d) -> p g d", g=2)
                    for g in range(2):
                        nc.vector.bn_stats(out=stats[:, g, :], in_=fg[:, g, :])
                    mv = lnsm.tile([128, 2], F32, name="mv", tag="mv")
                    nc.vector.bn_aggr(out=mv[:], in_=stats[:])
                    nc.scalar.activation(
                        out=mv[:, 1:2], in_=mv[:, 1:2], func=AF.Sqrt,
                        bias=eps_t[:], scale=1.0,
                    )
                    nc.vector.reciprocal(out=mv[:, 1:2], in_=mv[:, 1:2])
                    nc.vector.tensor_scalar(
                        out=ft[:], in0=ft[:],
                        scalar1=mv[:, 0:1], scalar2=mv[:, 1:2],
                        op0=ALU.subtract, op1=ALU.mult,
                    )
                    nc.vector.tensor_mul(out=ft[:], in0=ft[:], in1=ga_rep[:])
                    nc.vector.tensor_add(out=ft[:], in0=ft[:], in1=be_rep[:])
                    nc.sync.dma_start(out=out_o[128 * qt:128 * (qt + 1), :], in_=ft[:])

    nc.finalize()
    return nc


def get_program():
    if "nc" not in _PROGRAM_CACHE:
        _PROGRAM_CACHE["nc"] = _build_program()
    return _PROGRAM_CACHE["nc"]


def prep_in_maps(Q, K, V, Wq, bq, Wk, bk, Wv, bv, Wo, bo, gamma, beta):
    """Build the 8 per-core input maps (all values np.float32)."""
    f = np.float32
    boeff = (bo + bv @ Wo).astype(f).reshape(1, D)
    gamma_r = gamma.astype(f).reshape(1, D)
    beta_r = beta.astype(f).reshape(1, D)
    in_maps = []
    qT = [np.ascontiguousarray(Q[b].T, dtype=f) for b in range(B)]
    kT = [np.ascontiguousarray(K[b].T, dtype=f) for b in range(B)]
    vT = [np.ascontiguousarray(V[b].T, dtype=f) for b in range(B)]
    for c in range(N_CORES):
        b, j = c // 4, c % 4
        hs = HPC * DK * j
        in_maps.append({
            "qT": qT[b], "kT": kT[b], "vT": vT[b],
            "wq4": np.ascontiguousarray(Wq[:, hs:hs + HPC * DK], dtype=f),
            "wk4": np.ascontiguousarray(Wk[:, hs:hs + HPC * DK], dtype=f),
            "wv4": np.ascontiguousarray(Wv[:, hs:hs + HPC * DV], dtype=f),
            "wo4": np.ascontiguousarray(Wo[hs:hs + HPC * DV, :], dtype=f),
            "bq2": np.ascontiguousarray(bq[hs:hs + 256].reshape(2, 128).T, dtype=f),
            "bk2": np.ascontiguousarray(bk[hs:hs + 256].reshape(2, 128).T, dtype=f),
            "boeff": boeff, "gamma_r": gamma_r, "beta_r": beta_r,
            "qres": np.ascontiguousarray(Q[b, ROWS * j:ROWS * (j + 1), :], dtype=f),
        })
    return in_maps


def assemble(results):
    output = np.empty((B, S, D), dtype=np.float32)
    attn = np.empty((B, H, S, S), dtype=np.float32)
    for c in range(N_CORES):
        b, j = c // 4, c % 4
        output[b, ROWS * j:ROWS * (j + 1), :] = results[c]["out_o"]
        attn[b, HPC * j:HPC * (j + 1), :, :] = results[c]["attn_o"]
    return output, attn


def _numpy_reference(Q, K, V, attn_mask, Wq, bq, Wk, bk, Wv, bv, Wo, bo, gamma, beta):
    """Fallback for the (unused in practice) masked case."""
    Qs = (Q @ Wq + bq).reshape(B, S, H, DK).transpose(0, 2, 1, 3)
    Ks = (K @ Wk + bk).reshape(B, S, H, DK).transpose(0, 2, 1, 3)
    Vs = (V @ Wv + bv).reshape(B, S, H, DV).transpose(0, 2, 1, 3)
    scores = np.einsum("bhqd,bhkd->bhqk", Qs, Ks) / np.sqrt(DK).astype(np.float32)
    scores = np.where(attn_mask[:, None, :, :], np.float32(-1e9), scores)
    m = scores.max(axis=-1, keepdims=True)
    e = np.exp(scores - m)
    attn = e / e.sum(axis=-1, keepdims=True)
    ctx = np.einsum("bhqk,bhkd->bhqd", attn, Vs)
    ctx = ctx.transpose(0, 2, 1, 3).reshape(B, S, H * DV)
    out = ctx @ Wo + bo + Q
    mu = out.mean(axis=-1, keepdims=True)
    var = ((out - mu) ** 2).mean(axis=-1, keepdims=True)
    out = (out - mu) / np.sqrt(var + LN_EPS) * gamma + beta
    return out.astype(np.float32), attn.astype(np.float32)


def kernel(Q, K, V, attn_mask, Wq, bq, Wk, bk, Wv, bv, Wo, bo, gamma, beta):
    args = [np.asarray(x) for x in
            (Q, K, V, attn_mask, Wq, bq, Wk, bk, Wv, bv, Wo, bo, gamma, beta)]
    Q, K, V, attn_mask = args[:4]
    if np.asarray(attn_mask).any():
        return _numpy_reference(*args)
    nc = get_program()
    in_maps = prep_in_maps(Q, K, V, *args[4:])
    res = run_bass_kernel_spmd(nc, in_maps, core_ids=list(range(N_CORES)))
    return assemble(res.results)


if __name__ == "__main__":
    pass


# revision 11
# speedup vs baseline: 82.8136x; 82.8136x over previous
"""Multi-head attention (B=2, S=2048, D=1024, H=16, d_k=d_v=64) on 8 trn2 cores.

Sharding: core c handles batch b = c//4 and heads [4j, 4j+4) where j = c%4
(data parallel over B, tensor parallel over heads). Each core:

  1. Projects its 4 heads' Q/K/V in feature-major ("transposed") layout
     directly from host-pre-transposed Q^T/K^T/V^T inputs:
         QhT = Wq_h^T @ Q^T   [64, 2048]   (d_k on partitions)
     so both score layouts come straight off the tensor engine with no
     on-chip transposes.
  2. Pass 1 (k-major): scoresT[k,q] tiles -> exp on ACT -> AV matmuls with a
     ones-column appended to V, so PSUM row 64 accumulates the softmax
     denominator for free (no partition-direction reductions). The AV
     matmuls for k-chunk t are emitted after the score matmuls for chunk
     t+1, hiding the ACT exp latency from the in-order PE queue.
  3. Context is normalized in feature-major form and multiplied by this
     core's 256-row slice of Wo -> partial output [2048, 1024];
     ReduceScatter(add) over the 4 cores of the batch group sums the
     partials and hands each core its own 512-row chunk. The collective
     runs on the TOPSP/SDMA path and overlaps with pass 2.
  4. Pass 2 (q-major): recompute scores with Q as the stationary operand and
     apply exp(s/8 - log(denom)) in one ACT op (bias = -log denom, obtained
     by a tiny PE transpose of the denominator row) -> normalized attention
     probabilities written straight to DRAM in output layout.
  5. Bias + residual + LayerNorm run locally on the 512-row chunk; host
     concatenates the chunks.

ATTN_DT selects the attention-matmul operand dtype: float32r (single-pass
fp32 on the PE, ~4e-4 rel err) or bfloat16 (faster weight path, ~2e-3).
"""

import numpy as np

import concourse.bass as bass
import concourse.mybir as mybir
import concourse.tile as tile
from concourse import bacc
from concourse.bass_utils import run_bass_kernel_spmd
from concourse.masks import make_identity

F32 = mybir.dt.float32
F32R = mybir.dt.float32r
BF16 = mybir.dt.bfloat16
AF = mybir.ActivationFunctionType
ALU = mybir.AluOpType

B, S, D = 2, 2048, 1024
H, DK, DV = 16, 64, 64
HPC = 4              # heads per core
N_CORES = 8
ROWS = S // 4        # out rows owned per core (ReduceScatter chunk)
LN_EPS = 1e-5
REPLICA_GROUPS = [[0, 1, 2, 3], [4, 5, 6, 7]]

ATTN_DT = F32R       # dtype of QhT/KhT/Vh/exp operands on the PE

_PROGRAM_CACHE = {}


def _build_program(write_attn=True, attn_dt=None):
    if attn_dt is None:
        attn_dt = ATTN_DT
    nc = bacc.Bacc(None, target_bir_lowering=False)

    # ---- I/O ----------------------------------------------------------------
    qT = nc.dram_tensor("qT", [D, S], F32R, kind="ExternalInput")
    kT = nc.dram_tensor("kT", [D, S], F32R, kind="ExternalInput")
    vT = nc.dram_tensor("vT", [D, S], F32R, kind="ExternalInput")
    wq4 = nc.dram_tensor("wq4", [D, HPC * DK], F32R, kind="ExternalInput")
    wk4 = nc.dram_tensor("wk4", [D, HPC * DK], F32R, kind="ExternalInput")
    wv4 = nc.dram_tensor("wv4", [D, HPC * DV], F32R, kind="ExternalInput")
    wo4 = nc.dram_tensor("wo4", [HPC * DV, D], F32, kind="ExternalInput")
    bq2 = nc.dram_tensor("bq2", [128, 2], F32, kind="ExternalInput")
    bk2 = nc.dram_tensor("bk2", [128, 2], F32, kind="ExternalInput")
    boeff = nc.dram_tensor("boeff", [1, D], F32, kind="ExternalInput")
    gamma_r = nc.dram_tensor("gamma_r", [1, D], F32, kind="ExternalInput")
    beta_r = nc.dram_tensor("beta_r", [1, D], F32, kind="ExternalInput")
    qres = nc.dram_tensor("qres", [ROWS, D], F32, kind="ExternalInput")

    attn_shape = [HPC, S, S] if write_attn != "tiny" else [1, 128, S]
    attn_o = nc.dram_tensor("attn_o", attn_shape, F32, kind="ExternalOutput")
    out_o = nc.dram_tensor("out_o", [ROWS, D], F32, kind="ExternalOutput")

    NKT = S // 128   # 16 k tiles of 128
    NQB = S // 512   # 4 q blocks of 512
    NQT = S // 128   # 16 q tiles of 128

    with tile.TileContext(nc) as tc:
        with (
            tc.tile_pool(name="persist", bufs=1) as persist,
            tc.tile_pool(name="col_pool", bufs=HPC * (S // 128)) as col_pool,
            tc.tile_pool(name="dram", bufs=1, space="DRAM") as dram,
        ):
            # persistent SBUF state
            qhT = [persist.tile([128, S], attn_dt, name=f"qhT{p}") for p in range(2)]
            khT = [persist.tile([128, S], attn_dt, name=f"khT{p}") for p in range(2)]
            vh_aug = [persist.tile([128, NKT * (DV + 1)], attn_dt, name=f"vhaug{h}")
                      for h in range(HPC)]
            ctxn = [persist.tile([128, S], F32R, name=f"ctxn{p}") for p in range(2)]
            ident = persist.tile([128, 128], F32, name="ident")
            make_identity(nc, ident[:])
            ones64 = persist.tile([1, 64], F32, name="ones64")
            nc.vector.memset(ones64[:], 1.0)
            nldpad = persist.tile([128, 512], F32, name="nldpad")
            nc.vector.memset(nldpad[:], 0.0)
            onecol = persist.tile([128, 1], F32, name="onecol")
            nc.vector.memset(onecol[:], 1.0)

            bq_sb = persist.tile([128, 2], F32, name="bq_sb")
            bk_sb = persist.tile([128, 2], F32, name="bk_sb")
            nc.sync.dma_start(out=bq_sb[:], in_=bq2[:])
            nc.sync.dma_start(out=bk_sb[:], in_=bk2[:])

            # ones column (col 64 of each 65-wide chunk) of vh_aug
            for h in range(HPC):
                for kt in range(NKT):
                    nc.vector.tensor_copy(
                        vh_aug[h][:, (DV + 1) * kt + DV:(DV + 1) * kt + DV + 1],
                        onecol[:],
                    )

            cc_in = dram.tile([S, D], F32, name="cc_in")
            cc_out = dram.tile([ROWS, D], F32, name="cc_out")

            nld_cols = {}   # (h, qt) -> [128,1] tile holding -log denom

            with (
                tc.tile_pool(name="ps_big", bufs=2, space="PSUM") as ps_big,
                tc.tile_pool(name="ps_ctx", bufs=2, space="PSUM") as ps_ctx,
                tc.tile_pool(name="ps_misc", bufs=2, space="PSUM") as ps_misc,
            ):
                # ---- Phase A: projections ----------------------------------
                with (
                    tc.tile_pool(name="wpool", bufs=1) as wpool,
                    tc.tile_pool(name="inpool", bufs=9) as inpool,
                ):
                    wq_sb = wpool.tile([128, 8, HPC * DK], F32R, name="wq_sb")
                    wk_sb = wpool.tile([128, 8, HPC * DK], F32R, name="wk_sb")
                    wv_sb = wpool.tile([128, 8, HPC * DV], F32R, name="wv_sb")
                    nc.sync.dma_start(out=wq_sb[:], in_=wq4.rearrange("(t p) n -> p t n", p=128))
                    nc.sync.dma_start(out=wk_sb[:], in_=wk4.rearrange("(t p) n -> p t n", p=128))
                    nc.sync.dma_start(out=wv_sb[:], in_=wv4.rearrange("(t p) n -> p t n", p=128))

                    def project_qk(src_dram, w_sb, b_sb, dst_pair):
                        tiles = []
                        for t in range(8):
                            it = inpool.tile([128, S], F32R, name=f"in{t}", tag="intile")
                            nc.sync.dma_start(out=it[:], in_=src_dram[128 * t:128 * (t + 1), :])
                            tiles.append(it)
                        for pair in range(2):
                            for sb4 in range(NQB):
                                ps = ps_big.tile([128, 512], F32, name="projps", tag="bigps")
                                for t in range(8):
                                    nc.tensor.matmul(
                                        ps[:],
                                        lhsT=w_sb[:, t, 128 * pair:128 * (pair + 1)],
                                        rhs=tiles[t][:, 512 * sb4:512 * (sb4 + 1)],
                                        start=(t == 0), stop=(t == 7),
                                    )
                                nc.vector.tensor_scalar_add(
                                    out=dst_pair[pair][:, 512 * sb4:512 * (sb4 + 1)],
                                    in0=ps[:],
                                    scalar1=b_sb[:, pair:pair + 1],
                                )

                    project_qk(qT, wq_sb, bq_sb, qhT)
                    project_qk(kT, wk_sb, bk_sb, khT)

                    vtiles = []
                    for t in range(8):
                        it = inpool.tile([128, S], F32R, name=f"vin{t}", tag="intile")
                        nc.sync.dma_start(out=it[:], in_=vT[128 * t:128 * (t + 1), :])
                        vtiles.append(it)
                    for kt in range(NKT):
                        ps = ps_big.tile([128, HPC * DV], F32, name="vps", tag="bigps")
                        for t in range(8):
                            nc.tensor.matmul(
                                ps[:],
                                lhsT=vtiles[t][:, 128 * kt:128 * (kt + 1)],
                                rhs=wv_sb[:, t, :],
                                start=(t == 0), stop=(t == 7),
                            )
                        for h in range(HPC):
                            nc.vector.tensor_copy(
                                vh_aug[h][:, (DV + 1) * kt:(DV + 1) * kt + DV],
                                ps[:, DV * h:DV * (h + 1)],
                            )

                # ---- Phase B1: pass-1 attention (k-major) ------------------
                with (
                    tc.tile_pool(name="exp_pool", bufs=3) as exp_pool,
                    tc.tile_pool(name="sm_pool", bufs=4) as sm_pool,
                ):
                    for h in range(HPC):
                        p, off = h // 2, 64 * (h % 2)
                        for qb in range(NQB):
                            qsl = slice(512 * qb, 512 * (qb + 1))
                            ctx_ps = ps_ctx.tile([128, 512], F32, name="ctxps", tag="ctxps")
                            exs = [None] * (NKT // 2)

                            def emit_scores(kt2):
                                sc = ps_big.tile([128, 1024], F32, name="scps", tag="bigps")
                                for half in range(2):
                                    kt = 2 * kt2 + half
                                    nc.tensor.matmul(
                                        sc[:, 512 * half:512 * (half + 1)],
                                        lhsT=khT[p][off:off + 64, 128 * kt:128 * (kt + 1)],
                                        rhs=qhT[p][off:off + 64, qsl],
                                        start=True, stop=True,
                                    )
                                ex = exp_pool.tile([128, 1024], attn_dt, name="ex", tag="ex")
                                nc.scalar.activation(out=ex[:], in_=sc[:], func=AF.Exp, scale=0.125)
                                exs[kt2] = ex

                            def emit_av(kt2):
                                for half in range(2):
                                    kt = 2 * kt2 + half
                                    nc.tensor.matmul(
                                        ctx_ps[0:DV + 1, :],
                                        lhsT=vh_aug[h][:, (DV + 1) * kt:(DV + 1) * (kt + 1)],
                                        rhs=exs[kt2][:, 512 * half:512 * (half + 1)],
                                        start=(kt == 0), stop=(kt == NKT - 1),
                                    )

                            # software pipeline: scores(k+1) issued before av(k)
                            emit_scores(0)
                            for kt2 in range(1, NKT // 2):
                                emit_scores(kt2)
                                emit_av(kt2 - 1)
                            emit_av(NKT // 2 - 1)

                            # denominator handling
                            inv_d = sm_pool.tile([1, 512], F32, name="inv_d", tag="inv_d")
                            nc.vector.reciprocal(out=inv_d[:], in_=ctx_ps[DV:DV + 1, :])
                            nld = sm_pool.tile([1, 512], F32, name="nld", tag="nld")
                            nc.scalar.activation(out=nld[:], in_=inv_d[:], func=AF.Ln)
                            nc.vector.tensor_copy(nldpad[0:1, :], nld[:])
                            for qq in range(4):
                                trp = ps_misc.tile([128, 128], F32, name="trp", tag="miscps")
                                nc.tensor.transpose(trp[:], nldpad[:, 128 * qq:128 * (qq + 1)], ident[:])
                                nldc = col_pool.tile([128, 1], F32, name="nldc", tag="nldc")
                                nc.vector.tensor_copy(nldc[:], trp[:, 0:1])
                                nld_cols[(h, 4 * qb + qq)] = nldc
                            # normalize context (feature-major)
                            rep = ps_misc.tile([64, 512], F32, name="rep", tag="miscps")
                            nc.tensor.matmul(rep[:], lhsT=ones64[:], rhs=inv_d[:], start=True, stop=True)
                            rep_sb = sm_pool.tile([64, 512], F32, name="rep_sb", tag="rep_sb")
                            nc.vector.tensor_copy(rep_sb[:], rep[:])
                            nc.vector.tensor_mul(
                                ctxn[p][off:off + 64, qsl], ctx_ps[0:DV, :], rep_sb[:],
                            )

                    # ---- Phase C: partial out-projection + ReduceScatter ----
                    # (emitted before pass 2 so the collective overlaps it)
                    with (
                        tc.tile_pool(name="wo_pool", bufs=1) as wo_pool,
                        tc.tile_pool(name="stage_pool", bufs=3) as stage_pool,
                    ):
                        wo_sb = wo_pool.tile([128, 2, D], F32R, name="wo_sb")
                        nc.gpsimd.dma_start(out=wo_sb[:], in_=wo4.rearrange("(t p) n -> p t n", p=128))
                        for qt in range(NQT):
                            op = ps_big.tile([128, 1024], F32, name="opps", tag="bigps")
                            for dmb in range(2):
                                for t in range(2):
                                    nc.tensor.matmul(
                                        op[:, 512 * dmb:512 * (dmb + 1)],
                                        lhsT=ctxn[t][:, 128 * qt:128 * (qt + 1)],
                                        rhs=wo_sb[:, t, 512 * dmb:512 * (dmb + 1)],
                                        start=(t == 0), stop=(t == 1),
                                    )
                            stg = stage_pool.tile([128, D], F32, name="stg", tag="stg")
                            nc.vector.tensor_copy(stg[:], op[:])
                            nc.sync.dma_start(out=cc_in[128 * qt:128 * (qt + 1), :], in_=stg[:])
                        nc.gpsimd.collective_compute(
                            "ReduceScatter", ALU.add,
                            replica_groups=REPLICA_GROUPS,
                            ins=[cc_in.opt()], outs=[cc_out.opt()],
                        )

            # ---- Phase B2: pass-2 attention (q-major, normalized) ----------
            with (
                tc.tile_pool(name="ps2", bufs=2, space="PSUM") as ps2_pool,
                tc.tile_pool(name="attn_pool", bufs=3) as attn_pool,
            ):
                for h in range(HPC):
                    p, off = h // 2, 64 * (h % 2)
                    for qt in range(NQT):
                        s2 = ps2_pool.tile([128, 2048], F32, name="s2ps", tag="s2ps")
                        for kb in range(4):
                            nc.tensor.matmul(
                                s2[:, 512 * kb:512 * (kb + 1)],
                                lhsT=qhT[p][off:off + 64, 128 * qt:128 * (qt + 1)],
                                rhs=khT[p][off:off + 64, 512 * kb:512 * (kb + 1)],
                                start=True, stop=True,
                            )
                        at = attn_pool.tile([128, S], F32, name="at", tag="at")
                        nc.scalar.activation(
                            out=at[:], in_=s2[:], func=AF.Exp, scale=0.125,
                            bias=nld_cols[(h, qt)][:],
                        )
                        if write_attn is True:
                            nc.sync.dma_start(
                                out=attn_o[h, 128 * qt:128 * (qt + 1), :], in_=at[:],
                            )
                        elif h == 0 and qt == 0:
                            nc.sync.dma_start(out=attn_o[0, 0:128, :], in_=at[:])

            # ---- Phase D: bias + residual + LayerNorm ----------------------
            with (
                tc.tile_pool(name="fin_pool", bufs=4) as fin_pool,
                tc.tile_pool(name="ln_pool", bufs=1) as ln_pool,
                tc.tile_pool(name="lnsm", bufs=4) as lnsm,
            ):
                bo_rep = ln_pool.tile([128, D], F32, name="bo_rep")
                ga_rep = ln_pool.tile([128, D], F32, name="ga_rep")
                be_rep = ln_pool.tile([128, D], F32, name="be_rep")
                nc.gpsimd.dma_start(out=bo_rep[:], in_=boeff.ap().to_broadcast((128, D)))
                nc.gpsimd.dma_start(out=ga_rep[:], in_=gamma_r.ap().to_broadcast((128, D)))
                nc.gpsimd.dma_start(out=be_rep[:], in_=beta_r.ap().to_broadcast((128, D)))
                eps_t = ln_pool.tile([128, 1], F32, name="eps_t")
                nc.vector.memset(eps_t[:], LN_EPS)

                for qt in range(ROWS // 128):
                    ft = fin_pool.tile([128, D], F32, name="ft", tag="ft")
                    nc.sync.dma_start(out=ft[:], in_=cc_out[128 * qt:128 * (qt + 1), :])
                    rs = fin_pool.tile([128, D], F32, name="rs", tag="rs")
                    nc.sync.dma_start(out=rs[:], in_=qres[128 * qt:128 * (qt + 1), :])
                    nc.vector.tensor_add(out=ft[:], in0=ft[:], in1=bo_rep[:])
                    nc.vector.tensor_add(out=ft[:], in0=ft[:], in1=rs[:])
                    stats = lnsm.tile([128, 2, 6], F32, name="stats", tag="stats")
                    fg = ft.rearrange("p (g d) -> p g d", g=2)
                    for g in range(2):
                        nc.vector.bn_stats(out=stats[:, g, :], in_=fg[:, g, :])
                    mv = lnsm.tile([128, 2], F32, name="mv", tag="mv")
                    nc.vector.bn_aggr(out=mv[:], in_=stats[:])
                    nc.scalar.activation(
                        out=mv[:, 1:2], in_=mv[:, 1:2], func=AF.Sqrt,
                        bias=eps_t[:], scale=1.0,
                    )
                    nc.vector.reciprocal(out=mv[:, 1:2], in_=mv[:, 1:2])
                    nc.vector.tensor_scalar(
                        out=ft[:], in0=ft[:],
                        scalar1=mv[:, 0:1], scalar2=mv[:, 1:2],
                        op0=ALU.subtract, op1=ALU.mult,
                    )
                    nc.vector.tensor_mul(out=ft[:], in0=ft[:], in1=ga_rep[:])
                    nc.vector.tensor_add(out=ft[:], in0=ft[:], in1=be_rep[:])
                    nc.sync.dma_start(out=out_o[128 * qt:128 * (qt + 1), :], in_=ft[:])

    nc.finalize()
    return nc


def get_program():
    if "nc" not in _PROGRAM_CACHE:
        _PROGRAM_CACHE["nc"] = _build_program()
    return _PROGRAM_CACHE["nc"]


def prep_in_maps(Q, K, V, Wq, bq, Wk, bk, Wv, bv, Wo, bo, gamma, beta):
    """Build the 8 per-core input maps (all values np.float32)."""
    f = np.float32
    boeff = (bo + bv @ Wo).astype(f).reshape(1, D)
    gamma_r = gamma.astype(f).reshape(1, D)
    beta_r = beta.astype(f).reshape(1, D)
    in_maps = []
    qT = [np.ascontiguousarray(Q[b].T, dtype=f) for b in range(B)]
    kT = [np.ascontiguousarray(K[b].T, dtype=f) for b in range(B)]
    vT = [np.ascontiguousarray(V[b].T, dtype=f) for b in range(B)]
    for c in range(N_CORES):
        b, j = c // 4, c % 4
        hs = HPC * DK * j
        in_maps.append({
            "qT": qT[b], "kT": kT[b], "vT": vT[b],
            "wq4": np.ascontiguousarray(Wq[:, hs:hs + HPC * DK], dtype=f),
            "wk4": np.ascontiguousarray(Wk[:, hs:hs + HPC * DK], dtype=f),
            "wv4": np.ascontiguousarray(Wv[:, hs:hs + HPC * DV], dtype=f),
            "wo4": np.ascontiguousarray(Wo[hs:hs + HPC * DV, :], dtype=f),
            "bq2": np.ascontiguousarray(bq[hs:hs + 256].reshape(2, 128).T, dtype=f),
            "bk2": np.ascontiguousarray(bk[hs:hs + 256].reshape(2, 128).T, dtype=f),
            "boeff": boeff, "gamma_r": gamma_r, "beta_r": beta_r,
            "qres": np.ascontiguousarray(Q[b, ROWS * j:ROWS * (j + 1), :], dtype=f),
        })
    return in_maps


def assemble(results):
    output = np.empty((B, S, D), dtype=np.float32)
    attn = np.empty((B, H, S, S), dtype=np.float32)
    for c in range(N_CORES):
        b, j = c // 4, c % 4
        output[b, ROWS * j:ROWS * (j + 1), :] = results[c]["out_o"]
        attn[b, HPC * j:HPC * (j + 1), :, :] = results[c]["attn_o"]
    return output, attn


def _numpy_reference(Q, K, V, attn_mask, Wq, bq, Wk, bk, Wv, bv, Wo, bo, gamma, beta):
    """Fallback for the (unused in practice) masked case."""
    Qs = (Q @ Wq + bq).reshape(B, S, H, DK).transpose(0, 2, 1, 3)
    Ks = (K @ Wk + bk).reshape(B, S, H, DK).transpose(0, 2, 1, 3)
    Vs = (V @ Wv + bv).reshape(B, S, H, DV).transpose(0, 2, 1, 3)
    scores = np.einsum("bhqd,bhkd->bhqk", Qs, Ks) / np.sqrt(DK).astype(np.float32)
    scores = np.where(attn_mask[:, None, :, :], np.float32(-1e9), scores)
    m = scores.max(axis=-1, keepdims=True)
    e = np.exp(scores - m)
    attn = e / e.sum(axis=-1, keepdims=True)
    ctx = np.einsum("bhqk,bhkd->bhqd", attn, Vs)
    ctx = ctx.transpose(0, 2, 1, 3).reshape(B, S, H * DV)
    out = ctx @ Wo + bo + Q
    mu = out.mean(axis=-1, keepdims=True)
    var = ((out - mu) ** 2).mean(axis=-1, keepdims=True)
    out = (out - mu) / np.sqrt(var + LN_EPS) * gamma + beta
    return out.astype(np.float32), attn.astype(np.float32)


def kernel(Q, K, V, attn_mask, Wq, bq, Wk, bk, Wv, bv, Wo, bo, gamma, beta):
    args = [np.asarray(x) for x in
            (Q, K, V, attn_mask, Wq, bq, Wk, bk, Wv, bv, Wo, bo, gamma, beta)]
    Q, K, V, attn_mask = args[:4]
    if np.asarray(attn_mask).any():
        return _numpy_reference(*args)
    nc = get_program()
    in_maps = prep_in_maps(Q, K, V, *args[4:])
    res = run_bass_kernel_spmd(nc, in_maps, core_ids=list(range(N_CORES)))
    return assemble(res.results)


if __name__ == "__main__":
    pass


# revision 12
# speedup vs baseline: 83.5051x; 1.0084x over previous
"""Multi-head attention (B=2, S=2048, D=1024, H=16, d_k=d_v=64) on 8 trn2 cores.

Sharding: core c handles batch b = c//4 and heads [4j, 4j+4) where j = c%4
(data parallel over B, tensor parallel over heads). Each core:

  1. Projects its 4 heads' Q/K/V in feature-major ("transposed") layout
     directly from host-pre-transposed Q^T/K^T/V^T inputs:
         QhT = Wq_h^T @ Q^T   [64, 2048]   (d_k on partitions)
     so both score layouts come straight off the tensor engine with no
     on-chip transposes.
  2. Pass 1 (k-major): scoresT[k,q] tiles -> exp on ACT -> AV matmuls with a
     ones-column appended to V, so PSUM row 64 accumulates the softmax
     denominator for free (no partition-direction reductions). The AV
     matmuls for k-chunk t are emitted after the score matmuls for chunk
     t+1, hiding the ACT exp latency from the in-order PE queue.
  3. Context is normalized in feature-major form and multiplied by this
     core's 256-row slice of Wo -> partial output [2048, 1024];
     ReduceScatter(add) over the 4 cores of the batch group sums the
     partials and hands each core its own 512-row chunk. The collective
     runs on the TOPSP/SDMA path and overlaps with pass 2.
  4. Pass 2 (q-major): recompute scores with Q as the stationary operand and
     apply exp(s/8 - log(denom)) in one ACT op (bias = -log denom, obtained
     by a tiny PE transpose of the denominator row) -> normalized attention
     probabilities written straight to DRAM in output layout.
  5. Bias + residual + LayerNorm run locally on the 512-row chunk; host
     concatenates the chunks.

ATTN_DT selects the attention-matmul operand dtype: float32r (single-pass
fp32 on the PE, ~4e-4 rel err) or bfloat16 (faster weight path, ~2e-3).
"""

import numpy as np

import concourse.bass as bass
import concourse.mybir as mybir
import concourse.tile as tile
from concourse import bacc
from concourse.bass_utils import run_bass_kernel_spmd
from concourse.masks import make_identity

F32 = mybir.dt.float32
F32R = mybir.dt.float32r
BF16 = mybir.dt.bfloat16
AF = mybir.ActivationFunctionType
ALU = mybir.AluOpType

B, S, D = 2, 2048, 1024
H, DK, DV = 16, 64, 64
HPC = 4              # heads per core
N_CORES = 8
ROWS = S // 4        # out rows owned per core (ReduceScatter chunk)
LN_EPS = 1e-5
REPLICA_GROUPS = [[0, 1, 2, 3], [4, 5, 6, 7]]

ATTN_DT = F32R       # dtype of QhT/KhT/Vh/exp operands on the PE

_PROGRAM_CACHE = {}


def _build_program(write_attn=True, attn_dt=None):
    if attn_dt is None:
        attn_dt = ATTN_DT
    nc = bacc.Bacc(None, target_bir_lowering=False)

    # ---- I/O ----------------------------------------------------------------
    qT = nc.dram_tensor("qT", [D, S], F32R, kind="ExternalInput")
    kT = nc.dram_tensor("kT", [D, S], F32R, kind="ExternalInput")
    vT = nc.dram_tensor("vT", [D, S], F32R, kind="ExternalInput")
    wq4 = nc.dram_tensor("wq4", [D, HPC * DK], F32R, kind="ExternalInput")
    wk4 = nc.dram_tensor("wk4", [D, HPC * DK], F32R, kind="ExternalInput")
    wv4 = nc.dram_tensor("wv4", [D, HPC * DV], F32R, kind="ExternalInput")
    wo4 = nc.dram_tensor("wo4", [HPC * DV, D], F32, kind="ExternalInput")
    bq2 = nc.dram_tensor("bq2", [128, 2], F32, kind="ExternalInput")
    bk2 = nc.dram_tensor("bk2", [128, 2], F32, kind="ExternalInput")
    boeff = nc.dram_tensor("boeff", [1, D], F32, kind="ExternalInput")
    gamma_r = nc.dram_tensor("gamma_r", [1, D], F32, kind="ExternalInput")
    beta_r = nc.dram_tensor("beta_r", [1, D], F32, kind="ExternalInput")
    qres = nc.dram_tensor("qres", [ROWS, D], F32, kind="ExternalInput")

    attn_shape = [HPC, S, S] if write_attn != "tiny" else [1, 128, S]
    attn_o = nc.dram_tensor("attn_o", attn_shape, F32, kind="ExternalOutput")
    out_o = nc.dram_tensor("out_o", [ROWS, D], F32, kind="ExternalOutput")

    NKT = S // 128   # 16 k tiles of 128
    NQB = S // 512   # 4 q blocks of 512
    NQT = S // 128   # 16 q tiles of 128

    with tile.TileContext(nc) as tc:
        with (
            tc.tile_pool(name="persist", bufs=1) as persist,
            tc.tile_pool(name="col_pool", bufs=HPC * (S // 128)) as col_pool,
            tc.tile_pool(name="dram", bufs=1, space="DRAM") as dram,
        ):
            # persistent SBUF state
            qhT = [persist.tile([128, S], attn_dt, name=f"qhT{p}") for p in range(2)]
            khT = [persist.tile([128, S], attn_dt, name=f"khT{p}") for p in range(2)]
            vh_aug = [persist.tile([128, NKT * (DV + 1)], attn_dt, name=f"vhaug{h}")
                      for h in range(HPC)]
            ctxn = [persist.tile([128, S], F32R, name=f"ctxn{p}") for p in range(2)]
            ident = persist.tile([128, 128], F32, name="ident")
            make_identity(nc, ident[:])
            ones64 = persist.tile([1, 64], F32, name="ones64")
            nc.vector.memset(ones64[:], 1.0)
            nldpad = persist.tile([128, 512], F32, name="nldpad")
            nc.vector.memset(nldpad[:], 0.0)
            onecol = persist.tile([128, 1], F32, name="onecol")
            nc.vector.memset(onecol[:], 1.0)

            bq_sb = persist.tile([128, 2], F32, name="bq_sb")
            bk_sb = persist.tile([128, 2], F32, name="bk_sb")
            nc.sync.dma_start(out=bq_sb[:], in_=bq2[:])
            nc.sync.dma_start(out=bk_sb[:], in_=bk2[:])

            # ones column (col 64 of each 65-wide chunk) of vh_aug
            for h in range(HPC):
                for kt in range(NKT):
                    nc.vector.tensor_copy(
                        vh_aug[h][:, (DV + 1) * kt + DV:(DV + 1) * kt + DV + 1],
                        onecol[:],
                    )

            cc_in = [dram.tile([512, D], F32, name=f"cc_in{i}") for i in range(4)]
            cc_out = [dram.tile([128, D], F32, name=f"cc_out{i}") for i in range(4)]

            nld_cols = {}   # (h, qt) -> [128,1] tile holding -log denom

            with (
                tc.tile_pool(name="ps_big", bufs=2, space="PSUM") as ps_big,
                tc.tile_pool(name="ps_ctx", bufs=2, space="PSUM") as ps_ctx,
                tc.tile_pool(name="ps_misc", bufs=2, space="PSUM") as ps_misc,
            ):
                # ---- Phase A: projections ----------------------------------
                with (
                    tc.tile_pool(name="wpool", bufs=1) as wpool,
                    tc.tile_pool(name="inpool", bufs=9) as inpool,
                ):
                    wq_sb = wpool.tile([128, 8, HPC * DK], F32R, name="wq_sb")
                    wk_sb = wpool.tile([128, 8, HPC * DK], F32R, name="wk_sb")
                    wv_sb = wpool.tile([128, 8, HPC * DV], F32R, name="wv_sb")
                    nc.sync.dma_start(out=wq_sb[:], in_=wq4.rearrange("(t p) n -> p t n", p=128))
                    nc.sync.dma_start(out=wk_sb[:], in_=wk4.rearrange("(t p) n -> p t n", p=128))
                    nc.sync.dma_start(out=wv_sb[:], in_=wv4.rearrange("(t p) n -> p t n", p=128))

                    def project_qk(src_dram, w_sb, b_sb, dst_pair):
                        tiles = []
                        for t in range(8):
                            it = inpool.tile([128, S], F32R, name=f"in{t}", tag="intile")
                            nc.sync.dma_start(out=it[:], in_=src_dram[128 * t:128 * (t + 1), :])
                            tiles.append(it)
                        for pair in range(2):
                            for sb4 in range(NQB):
                                ps = ps_big.tile([128, 512], F32, name="projps", tag="bigps")
                                for t in range(8):
                                    nc.tensor.matmul(
                                        ps[:],
                                        lhsT=w_sb[:, t, 128 * pair:128 * (pair + 1)],
                                        rhs=tiles[t][:, 512 * sb4:512 * (sb4 + 1)],
                                        start=(t == 0), stop=(t == 7),
                                    )
                                nc.vector.tensor_scalar_add(
                                    out=dst_pair[pair][:, 512 * sb4:512 * (sb4 + 1)],
                                    in0=ps[:],
                                    scalar1=b_sb[:, pair:pair + 1],
                                )

                    project_qk(qT, wq_sb, bq_sb, qhT)
                    project_qk(kT, wk_sb, bk_sb, khT)

                    vtiles = []
                    for t in range(8):
                        it = inpool.tile([128, S], F32R, name=f"vin{t}", tag="intile")
                        nc.sync.dma_start(out=it[:], in_=vT[128 * t:128 * (t + 1), :])
                        vtiles.append(it)
                    for kt in range(NKT):
                        ps = ps_big.tile([128, HPC * DV], F32, name="vps", tag="bigps")
                        for t in range(8):
                            nc.tensor.matmul(
                                ps[:],
                                lhsT=vtiles[t][:, 128 * kt:128 * (kt + 1)],
                                rhs=wv_sb[:, t, :],
                                start=(t == 0), stop=(t == 7),
                            )
                        for h in range(HPC):
                            nc.vector.tensor_copy(
                                vh_aug[h][:, (DV + 1) * kt:(DV + 1) * kt + DV],
                                ps[:, DV * h:DV * (h + 1)],
                            )

                # ---- Phases B/C interleaved per q-block ---------------------
                with (
                    tc.tile_pool(name="exp_pool", bufs=3) as exp_pool,
                    tc.tile_pool(name="sm_pool", bufs=4) as sm_pool,
                    tc.tile_pool(name="wo_pool", bufs=1) as wo_pool,
                    tc.tile_pool(name="stage_pool", bufs=3) as stage_pool,
                    tc.tile_pool(name="attn_pool", bufs=3) as attn_pool,
                    tc.tile_pool(name="fin_pool", bufs=4) as fin_pool,
                    tc.tile_pool(name="ln_pool", bufs=1) as ln_pool,
                    tc.tile_pool(name="lnsm", bufs=4) as lnsm,
                ):
                    wo_sb = wo_pool.tile([128, 2, D], F32R, name="wo_sb")
                    nc.gpsimd.dma_start(out=wo_sb[:], in_=wo4.rearrange("(t p) n -> p t n", p=128))
                    bo_rep = ln_pool.tile([128, D], F32, name="bo_rep")
                    ga_rep = ln_pool.tile([128, D], F32, name="ga_rep")
                    be_rep = ln_pool.tile([128, D], F32, name="be_rep")
                    nc.gpsimd.dma_start(out=bo_rep[:], in_=boeff.ap().to_broadcast((128, D)))
                    nc.gpsimd.dma_start(out=ga_rep[:], in_=gamma_r.ap().to_broadcast((128, D)))
                    nc.gpsimd.dma_start(out=be_rep[:], in_=beta_r.ap().to_broadcast((128, D)))
                    eps_t = ln_pool.tile([128, 1], F32, name="eps_t")
                    nc.vector.memset(eps_t[:], LN_EPS)

                    def emit_p1(h, qb):
                        p, off = h // 2, 64 * (h % 2)
                        qsl = slice(512 * qb, 512 * (qb + 1))
                        ctx_ps = ps_ctx.tile([128, 512], F32, name="ctxps", tag="ctxps")
                        exs = [None] * (NKT // 2)

                        def emit_scores(kt2):
                            sc = ps_big.tile([128, 1024], F32, name="scps", tag="bigps")
                            for half in range(2):
                                kt = 2 * kt2 + half
                                nc.tensor.matmul(
                                    sc[:, 512 * half:512 * (half + 1)],
                                    lhsT=khT[p][off:off + 64, 128 * kt:128 * (kt + 1)],
                                    rhs=qhT[p][off:off + 64, qsl],
                                    start=True, stop=True,
                                )
                            ex = exp_pool.tile([128, 1024], attn_dt, name="ex", tag="ex")
                            nc.scalar.activation(out=ex[:], in_=sc[:], func=AF.Exp, scale=0.125)
                            exs[kt2] = ex

                        def emit_av(kt2):
                            for half in range(2):
                                kt = 2 * kt2 + half
                                nc.tensor.matmul(
                                    ctx_ps[0:DV + 1, :],
                                    lhsT=vh_aug[h][:, (DV + 1) * kt:(DV + 1) * (kt + 1)],
                                    rhs=exs[kt2][:, 512 * half:512 * (half + 1)],
                                    start=(kt == 0), stop=(kt == NKT - 1),
                                )

                        emit_scores(0)
                        for kt2 in range(1, NKT // 2):
                            emit_scores(kt2)
                            emit_av(kt2 - 1)
                        emit_av(NKT // 2 - 1)

                        inv_d = sm_pool.tile([1, 512], F32, name="inv_d", tag="inv_d")
                        nc.vector.reciprocal(out=inv_d[:], in_=ctx_ps[DV:DV + 1, :])
                        nld = sm_pool.tile([1, 512], F32, name="nld", tag="nld")
                        nc.scalar.activation(out=nld[:], in_=inv_d[:], func=AF.Ln)
                        nc.vector.tensor_copy(nldpad[0:1, :], nld[:])
                        for qq in range(4):
                            trp = ps_misc.tile([128, 128], F32, name="trp", tag="miscps")
                            nc.tensor.transpose(trp[:], nldpad[:, 128 * qq:128 * (qq + 1)], ident[:])
                            nldc = col_pool.tile([128, 1], F32, name="nldc", tag="nldc")
                            nc.vector.tensor_copy(nldc[:], trp[:, 0:1])
                            nld_cols[(h, 4 * qb + qq)] = nldc
                        rep = ps_misc.tile([64, 512], F32, name="rep", tag="miscps")
                        nc.tensor.matmul(rep[:], lhsT=ones64[:], rhs=inv_d[:], start=True, stop=True)
                        rep_sb = sm_pool.tile([64, 512], F32, name="rep_sb", tag="rep_sb")
                        nc.vector.tensor_copy(rep_sb[:], rep[:])
                        nc.vector.tensor_mul(
                            ctxn[p][off:off + 64, qsl], ctx_ps[0:DV, :], rep_sb[:],
                        )

                    def emit_outproj_rs(qb):
                        for qtl in range(4):
                            qt = 4 * qb + qtl
                            op = ps_big.tile([128, 1024], F32, name="opps", tag="bigps")
                            for dmb in range(2):
                                for t in range(2):
                                    nc.tensor.matmul(
                                        op[:, 512 * dmb:512 * (dmb + 1)],
                                        lhsT=ctxn[t][:, 128 * qt:128 * (qt + 1)],
                                        rhs=wo_sb[:, t, 512 * dmb:512 * (dmb + 1)],
                                        start=(t == 0), stop=(t == 1),
                                    )
                            stg = stage_pool.tile([128, D], F32, name="stg", tag="stg")
                            nc.vector.tensor_copy(stg[:], op[:])
                            nc.sync.dma_start(out=cc_in[qb][128 * qtl:128 * (qtl + 1), :], in_=stg[:])
                        nc.gpsimd.collective_compute(
                            "ReduceScatter", ALU.add,
                            replica_groups=REPLICA_GROUPS,
                            ins=[cc_in[qb].opt()], outs=[cc_out[qb].opt()],
                        )

                    def emit_p2(h, qt):
                        p, off = h // 2, 64 * (h % 2)
                        at = attn_pool.tile([128, S], F32, name="at", tag="at")
                        for half2 in range(2):
                            s2 = ps_big.tile([128, 1024], F32, name="s2ps", tag="bigps")
                            for kb in range(2):
                                ko = 1024 * half2 + 512 * kb
                                nc.tensor.matmul(
                                    s2[:, 512 * kb:512 * (kb + 1)],
                                    lhsT=qhT[p][off:off + 64, 128 * qt:128 * (qt + 1)],
                                    rhs=khT[p][off:off + 64, ko:ko + 512],
                                    start=True, stop=True,
                                )
                            nc.scalar.activation(
                                out=at[:, 1024 * half2:1024 * (half2 + 1)],
                                in_=s2[:], func=AF.Exp, scale=0.125,
                                bias=nld_cols[(h, qt)][:],
                            )
                        if write_attn is True:
                            nc.sync.dma_start(
                                out=attn_o[h, 128 * qt:128 * (qt + 1), :], in_=at[:],
                            )
                        elif h == 0 and qt == 0:
                            nc.sync.dma_start(out=attn_o[0, 0:128, :], in_=at[:])

                    def emit_ln(qb):
                        ft = fin_pool.tile([128, D], F32, name="ft", tag="ft")
                        nc.sync.dma_start(out=ft[:], in_=cc_out[qb][:, :])
                        rs = fin_pool.tile([128, D], F32, name="rs", tag="rs")
                        nc.sync.dma_start(out=rs[:], in_=qres[128 * qb:128 * (qb + 1), :])
                        nc.vector.tensor_add(out=ft[:], in0=ft[:], in1=bo_rep[:])
                        nc.vector.tensor_add(out=ft[:], in0=ft[:], in1=rs[:])
                        stats = lnsm.tile([128, 2, 6], F32, name="stats", tag="stats")
                        fg = ft.rearrange("p (g d) -> p g d", g=2)
                        for g in range(2):
                            nc.vector.bn_stats(out=stats[:, g, :], in_=fg[:, g, :])
                        mv = lnsm.tile([128, 2], F32, name="mv", tag="mv")
                        nc.vector.bn_aggr(out=mv[:], in_=stats[:])
                        nc.scalar.activation(
                            out=mv[:, 1:2], in_=mv[:, 1:2], func=AF.Sqrt,
                            bias=eps_t[:], scale=1.0,
                        )
                        nc.vector.reciprocal(out=mv[:, 1:2], in_=mv[:, 1:2])
                        nc.vector.tensor_scalar(
                            out=ft[:], in0=ft[:],
                            scalar1=mv[:, 0:1], scalar2=mv[:, 1:2],
                            op0=ALU.subtract, op1=ALU.mult,
                        )
                        nc.vector.tensor_mul(out=ft[:], in0=ft[:], in1=ga_rep[:])
                        nc.vector.tensor_add(out=ft[:], in0=ft[:], in1=be_rep[:])
                        nc.sync.dma_start(out=out_o[128 * qb:128 * (qb + 1), :], in_=ft[:])

                    for qb in range(NQB):
                        for h in range(HPC):
                            emit_p1(h, qb)
                        emit_outproj_rs(qb)
                        for h in range(HPC):
                            for qtl in range(4):
                                emit_p2(h, 4 * qb + qtl)
                        emit_ln(qb)

    nc.finalize()
    return nc


def get_program():
    if "nc" not in _PROGRAM_CACHE:
        _PROGRAM_CACHE["nc"] = _build_program()
    return _PROGRAM_CACHE["nc"]


def prep_in_maps(Q, K, V, Wq, bq, Wk, bk, Wv, bv, Wo, bo, gamma, beta):
    """Build the 8 per-core input maps (all values np.float32)."""
    f = np.float32
    boeff = (bo + bv @ Wo).astype(f).reshape(1, D)
    gamma_r = gamma.astype(f).reshape(1, D)
    beta_r = beta.astype(f).reshape(1, D)
    in_maps = []
    qT = [np.ascontiguousarray(Q[b].T, dtype=f) for b in range(B)]
    kT = [np.ascontiguousarray(K[b].T, dtype=f) for b in range(B)]
    vT = [np.ascontiguousarray(V[b].T, dtype=f) for b in range(B)]
    for c in range(N_CORES):
        b, j = c // 4, c % 4
        hs = HPC * DK * j
        in_maps.append({
            "qT": qT[b], "kT": kT[b], "vT": vT[b],
            "wq4": np.ascontiguousarray(Wq[:, hs:hs + HPC * DK], dtype=f),
            "wk4": np.ascontiguousarray(Wk[:, hs:hs + HPC * DK], dtype=f),
            "wv4": np.ascontiguousarray(Wv[:, hs:hs + HPC * DV], dtype=f),
            "wo4": np.ascontiguousarray(Wo[hs:hs + HPC * DV, :], dtype=f),
            "bq2": np.ascontiguousarray(bq[hs:hs + 256].reshape(2, 128).T, dtype=f),
            "bk2": np.ascontiguousarray(bk[hs:hs + 256].reshape(2, 128).T, dtype=f),
            "boeff": boeff, "gamma_r": gamma_r, "beta_r": beta_r,
            "qres": np.ascontiguousarray(
                np.concatenate([Q[b, 512 * qb + 128 * j:512 * qb + 128 * (j + 1), :]
                                for qb in range(4)], axis=0), dtype=f),
        })
    return in_maps


def assemble(results):
    output = np.empty((B, S, D), dtype=np.float32)
    attn = np.empty((B, H, S, S), dtype=np.float32)
    for c in range(N_CORES):
        b, j = c // 4, c % 4
        for qb in range(4):
            output[b, 512 * qb + 128 * j:512 * qb + 128 * (j + 1), :] = \
                results[c]["out_o"][128 * qb:128 * (qb + 1)]
        attn[b, HPC * j:HPC * (j + 1), :, :] = results[c]["attn_o"]
    return output, attn


def _numpy_reference(Q, K, V, attn_mask, Wq, bq, Wk, bk, Wv, bv, Wo, bo, gamma, beta):
    """Fallback for the (unused in practice) masked case."""
    Qs = (Q @ Wq + bq).reshape(B, S, H, DK).transpose(0, 2, 1, 3)
    Ks = (K @ Wk + bk).reshape(B, S, H, DK).transpose(0, 2, 1, 3)
    Vs = (V @ Wv + bv).reshape(B, S, H, DV).transpose(0, 2, 1, 3)
    scores = np.einsum("bhqd,bhkd->bhqk", Qs, Ks) / np.sqrt(DK).astype(np.float32)
    scores = np.where(attn_mask[:, None, :, :], np.float32(-1e9), scores)
    m = scores.max(axis=-1, keepdims=True)
    e = np.exp(scores - m)
    attn = e / e.sum(axis=-1, keepdims=True)
    ctx = np.einsum("bhqk,bhkd->bhqd", attn, Vs)
    ctx = ctx.transpose(0, 2, 1, 3).reshape(B, S, H * DV)
    out = ctx @ Wo + bo + Q
    mu = out.mean(axis=-1, keepdims=True)
    var = ((out - mu) ** 2).mean(axis=-1, keepdims=True)
    out = (out - mu) / np.sqrt(var + LN_EPS) * gamma + beta
    return out.astype(np.float32), attn.astype(np.float32)


def kernel(Q, K, V, attn_mask, Wq, bq, Wk, bk, Wv, bv, Wo, bo, gamma, beta):
    args = [np.asarray(x) for x in
            (Q, K, V, attn_mask, Wq, bq, Wk, bk, Wv, bv, Wo, bo, gamma, beta)]
    Q, K, V, attn_mask = args[:4]
    if np.asarray(attn_mask).any():
        return _numpy_reference(*args)
    nc = get_program()
    in_maps = prep_in_maps(Q, K, V, *args[4:])
    res = run_bass_kernel_spmd(nc, in_maps, core_ids=list(range(N_CORES)))
    return assemble(res.results)


if __name__ == "__main__":
    pass
